# Initial kernel scaffold
#
"""Bidirectional LSTM (TF BasicLSTMCell semantics) on 8 Trainium2 NeuronCores.

Problem: x [64, 128, 512], per-direction W [1024, 2048], b [2048].
out [64, 128, 1024] = concat(h_fw, h_bw) over a T=128 sequential scan.

Sharding: 2 (direction) x 4 (batch quarters) = 8 cores, B_local = 16.
Every core runs the SAME program; direction is handled host-side by
time-reversing x (and the returned outputs) for the backward cores and
binding W_bw instead of W_fw.

Per-core program:
  phase 1:  G^T = Wx^T @ x^T + b   (one large matmul over all T*B columns,
            hidden/gate-major layout), kept fp16 in SBUF.
  scan:     for t: z^T = Whh_tiles^T @ h^T (64 accumulating 128x128 matmuls,
            fp16 weights -> fast weight load), z += G[t] (DVE),
            gates on ACT (sigmoid/tanh), c/h elementwise on DVE.
  output:   h stored hidden-major [512, T*B] in SBUF, 4 direct DMAs out;
            host does the final cheap transpose in numpy.
"""

import os
import sys

import numpy as np

for _p in ("/opt/trn_rl_repo", "/root/.axon_site/_ro/trn_rl_repo"):
    if os.path.isdir(_p) and _p not in sys.path:
        sys.path.insert(0, _p)

from contextlib import ExitStack

import concourse.bass as bass
import concourse.mybir as mybir
import concourse.tile as tile
from concourse.masks import make_identity

F32 = mybir.dt.float32
F16 = mybir.dt.float16
AF = mybir.ActivationFunctionType

B_FULL = 64
B_LOC = 16  # batch per core
T = 128
F = 512
H = 512
NG = 4 * H  # 2048 gate columns
KT = 4      # 128-row contraction tiles over F or H
MT = 16     # 128-col gate tiles
FORGET_BIAS = 1.0


def build_nc(t_steps: int = T) -> bass.Bass:
    tb = t_steps * B_LOC
    nch = tb // 128

    nc = bass.Bass("TRN2", target_bir_lowering=False, debug=False)
    x_d = nc.dram_tensor("x_tb", [tb, F], F32, kind="ExternalInput").ap()
    wx_d = nc.dram_tensor("wx", [F, NG], F32, kind="ExternalInput").ap()
    whh_d = nc.dram_tensor("whh", [H, NG], F16, kind="ExternalInput").ap()
    bias_d = nc.dram_tensor("bias", [128, MT], F32, kind="ExternalInput").ap()
    y_d = nc.dram_tensor("y", [H, tb], F32, kind="ExternalOutput").ap()

    with ExitStack() as ctx:
        tc = ctx.enter_context(tile.TileContext(nc))
        const = ctx.enter_context(tc.tile_pool(name="const", bufs=1))
        wx_sb = const.tile([128, KT * NG], F32, tag="wx")     # col = k*NG + m*128 + j
        whh_sb = const.tile([128, KT * NG], F16, tag="whh")   # col = k*NG + m*128 + j
        g_sb = const.tile([128, MT * tb], F16, tag="g")       # col = m*tb + t*16 + b
        hall = const.tile([128, KT * tb], F32, tag="hall")    # col = k*tb + t*16 + b
        bias_sb = const.tile([128, MT], F32, tag="bias")
        ident = const.tile([128, 128], F32, tag="ident")
        c_sb = const.tile([128, KT * B_LOC], F32, tag="c")

        make_identity(nc, ident[:])
        nc.vector.memset(c_sb[:], 0.0)

        for k in range(KT):
            nc.sync.dma_start(wx_sb[:, k * NG:(k + 1) * NG], wx_d[k * 128:(k + 1) * 128, :])
            nc.sync.dma_start(whh_sb[:, k * NG:(k + 1) * NG], whh_d[k * 128:(k + 1) * 128, :])
        nc.sync.dma_start(bias_sb[:], bias_d[:, :])

        # ---------- phase 1: G^T[m*128+p, (t,b)] = sum_f Wx[f, gate] x[(t,b), f] + b
        with tc.tile_pool(name="xrow", bufs=3) as xrow_p, \
             tc.tile_pool(name="xT", bufs=2) as xT_p, \
             tc.tile_pool(name="ptr", bufs=2, space="PSUM") as pt_p, \
             tc.tile_pool(name="p1", bufs=4, space="PSUM") as p1_p:
            for c in range(nch):
                xrow = xrow_p.tile([128, F], F32, tag="xrow")
                nc.sync.dma_start(xrow[:], x_d[c * 128:(c + 1) * 128, :])
                xT = xT_p.tile([128, F], F32, tag="xT")
                for k in range(KT):
                    pt = pt_p.tile([128, 128], F32, tag="pt")
                    nc.tensor.transpose(pt[:], xrow[:, k * 128:(k + 1) * 128], ident[:])
                    nc.vector.tensor_copy(xT[:, k * 128:(k + 1) * 128], pt[:])
                for g in range(4):
                    ps = p1_p.tile([128, 512], F32, tag="p1")
                    for mi in range(4):
                        m = 4 * g + mi
                        for k in range(KT):
                            nc.tensor.matmul(
                                ps[:, mi * 128:(mi + 1) * 128],
                                wx_sb[:, k * NG + m * 128: k * NG + (m + 1) * 128],
                                xT[:, k * 128:(k + 1) * 128],
                                start=(k == 0), stop=(k == KT - 1),
                            )
                    for mi in range(4):
                        m = 4 * g + mi
                        nc.vector.tensor_scalar_add(
                            g_sb[:, m * tb + c * 128: m * tb + (c + 1) * 128],
                            ps[:, mi * 128:(mi + 1) * 128],
                            bias_sb[:, m:m + 1],
                        )

        # ---------- scan
        g3 = g_sb[:].rearrange("p (m c) -> p m c", m=MT)
        h3 = hall[:].rearrange("p (k c) -> p k c", k=KT)
        ORDER = (2, 0, 1, 3)  # f, i, j, o: f/i/j early so c-chain overlaps o's matmuls
        with tc.tile_pool(name="sps", bufs=8, space="PSUM") as sps_p, \
             tc.tile_pool(name="zt", bufs=8) as z_p, \
             tc.tile_pool(name="act", bufs=3) as a_p, \
             tc.tile_pool(name="h16", bufs=3) as h_p, \
             tc.tile_pool(name="tmp", bufs=3) as tmp_p:
            hprev = h_p.tile([128, KT * B_LOC], F16, tag="h")
            nc.vector.memset(hprev[:], 0.0)
            for t in range(t_steps):
                zs = {}
                for g in ORDER:
                    ps = sps_p.tile([128, 4 * B_LOC], F32, tag="sps")
                    for mi in range(4):
                        m = 4 * g + mi
                        for k in range(KT):
                            nc.tensor.matmul(
                                ps[:, mi * B_LOC:(mi + 1) * B_LOC],
                                whh_sb[:, k * NG + m * 128: k * NG + (m + 1) * 128],
                                hprev[:, k * B_LOC:(k + 1) * B_LOC],
                                start=(k == 0), stop=(k == KT - 1),
                            )
                    z = z_p.tile([128, 4 * B_LOC], F32, tag="z%d" % g)
                    nc.vector.tensor_add(
                        z[:].rearrange("p (m c) -> p m c", m=4),
                        ps[:].rearrange("p (m c) -> p m c", m=4),
                        g3[:, 4 * g:4 * g + 4, t * B_LOC:(t + 1) * B_LOC],
                    )
                    zs[g] = z
                sig_f = a_p.tile([128, 4 * B_LOC], F32, tag="sigf")
                nc.scalar.activation(sig_f[:], zs[2][:], AF.Sigmoid, bias=FORGET_BIAS)
                sig_i = a_p.tile([128, 4 * B_LOC], F32, tag="sigi")
                nc.scalar.activation(sig_i[:], zs[0][:], AF.Sigmoid)
                tan_j = a_p.tile([128, 4 * B_LOC], F32, tag="tanj")
                nc.scalar.activation(tan_j[:], zs[1][:], AF.Tanh)
                sig_o = a_p.tile([128, 4 * B_LOC], F32, tag="sigo")
                nc.scalar.activation(sig_o[:], zs[3][:], AF.Sigmoid)

                tmp = tmp_p.tile([128, 4 * B_LOC], F32, tag="tmp")
                nc.vector.tensor_mul(tmp[:], sig_i[:], tan_j[:])
                nc.vector.tensor_mul(c_sb[:], c_sb[:], sig_f[:])
                nc.vector.tensor_add(c_sb[:], c_sb[:], tmp[:])
                tan_c = a_p.tile([128, 4 * B_LOC], F32, tag="tanc")
                nc.scalar.activation(tan_c[:], c_sb[:], AF.Tanh)

                hv = h3[:, :, t * B_LOC:(t + 1) * B_LOC]
                nc.vector.tensor_mul(
                    hv,
                    tan_c[:].rearrange("p (k c) -> p k c", k=KT),
                    sig_o[:].rearrange("p (k c) -> p k c", k=KT),
                )
                hnew = h_p.tile([128, KT * B_LOC], F16, tag="h")
                nc.vector.tensor_copy(hnew[:].rearrange("p (k c) -> p k c", k=KT), hv)
                hprev = hnew

        for k in range(KT):
            nc.sync.dma_start(y_d[k * 128:(k + 1) * 128, :], hall[:, k * tb:(k + 1) * tb])

    return nc


_BUILT: bass.Bass | None = None


def _get_built() -> bass.Bass:
    global _BUILT
    if _BUILT is None:
        _BUILT = build_nc(T)
    return _BUILT


def make_in_maps(x, W_fw, b_fw, W_bw, b_bw, t_steps: int = T):
    x = np.asarray(x, np.float32)
    in_maps = []
    for d, (Wd, bd) in enumerate(((W_fw, b_fw), (W_bw, b_bw))):
        Wd = np.asarray(Wd, np.float32)
        wx = np.ascontiguousarray(Wd[:F])
        whh = np.ascontiguousarray(Wd[F:]).astype(np.float16)
        bias = np.ascontiguousarray(np.asarray(bd, np.float32).reshape(MT, 128).T)
        for g in range(4):
            xg = x[g * B_LOC:(g + 1) * B_LOC, :t_steps]
            if d == 1:
                xg = xg[:, ::-1, :]
            x_tb = np.ascontiguousarray(
                xg.transpose(1, 0, 2).reshape(t_steps * B_LOC, F)
            )
            in_maps.append({"x_tb": x_tb, "wx": wx, "whh": whh, "bias": bias})
    return in_maps


def assemble_out(results, t_steps: int = T):
    out = np.empty((B_FULL, t_steps, 2 * H), np.float32)
    for idx, r in enumerate(results):
        d, g = divmod(idx, 4)
        h = r["y"].reshape(H, t_steps, B_LOC).transpose(2, 1, 0)  # [16, T, 512]
        if d == 1:
            h = h[:, ::-1, :]
        out[g * B_LOC:(g + 1) * B_LOC, :, d * H:(d + 1) * H] = h
    return out


def kernel(x, W_fw, b_fw, W_bw, b_bw):
    from concourse.bass_utils import run_bass_kernel_spmd

    nc = _get_built()
    in_maps = make_in_maps(x, W_fw, b_fw, W_bw, b_bw)
    res = run_bass_kernel_spmd(nc, in_maps, core_ids=list(range(8)))
    return assemble_out(res.results)


# revision 14
# speedup vs baseline: 2.1506x; 2.1506x over previous
"""Bidirectional LSTM (TF BasicLSTMCell semantics) on 8 Trainium2 NeuronCores.

Problem: x [64, 128, 512], per-direction W [1024, 2048], b [2048].
out [64, 128, 1024] = concat(h_fw, h_bw) over a T=128 sequential scan.

Sharding: 2 (direction) x 4 (batch quarters) = 8 cores, B_local = 16.
Every core runs the SAME program; direction is handled host-side by
time-reversing x (and the returned outputs) for the backward cores and
binding W_bw instead of W_fw.

Per-core program:
  phase 1:  G^T = Wx^T @ x^T + b (+1 on the f gate)  -- one large matmul over
            all T*B columns in 512-col groups, kept fp16 in SBUF.
  scan:     for t: z^T = Whh^T @ h^T as 64 accumulating 128x128 matmuls
            (fp16 weights -> fast weight load).  The hidden dim is split in
            two halves: each half's gates/c/h are computed while the other
            half's matmuls run, and h is produced as four fp16 quarter tiles
            so step t+1's k-quarter matmuls start as soon as quarter k of
            h(t) exists.  i/f/o share one merged Sigmoid per half.
  output:   h stored hidden-major [512, T*B] fp32 in SBUF, DMA'd out;
            host does the final cheap transpose in numpy.
"""

import os
import sys

import numpy as np

for _p in ("/opt/trn_rl_repo", "/root/.axon_site/_ro/trn_rl_repo"):
    if os.path.isdir(_p) and _p not in sys.path:
        sys.path.insert(0, _p)

from contextlib import ExitStack

import concourse.bass as bass
import concourse.mybir as mybir
import concourse.tile as tile
from concourse import bacc

F32 = mybir.dt.float32
F16 = mybir.dt.float16
AF = mybir.ActivationFunctionType

B_FULL = 64
B_LOC = 16  # batch per core
T = 128
F = 512
H = 512
NG = 4 * H  # 2048 gate columns
KT = 4      # 128-row contraction tiles over F or H
MT = 16     # 128-col gate tiles
FORGET_BIAS = 1.0


def build_nc(t_steps: int = T, repeat: int = 1) -> bass.Bass:
    tb = t_steps * B_LOC

    nc = bacc.Bacc("TRN2", target_bir_lowering=False, debug=False)
    x_d = nc.dram_tensor("xT", [F, tb], F32, kind="ExternalInput").ap()
    wx_d = nc.dram_tensor("wx", [F, NG], F32, kind="ExternalInput").ap()
    whh_d = nc.dram_tensor("whh", [H, NG], F16, kind="ExternalInput").ap()
    bias_d = nc.dram_tensor("bias", [128, MT], F32, kind="ExternalInput").ap()
    y_d = nc.dram_tensor("y", [H, tb], F32, kind="ExternalOutput").ap()

    with ExitStack() as ctx:
        tc = ctx.enter_context(tile.TileContext(nc))
        const = ctx.enter_context(tc.tile_pool(name="const", bufs=1))
        wx_sb = const.tile([128, KT * NG], F32, tag="wx")     # col = k*NG + m*128 + j
        whh_sb = const.tile([128, KT * NG], F16, tag="whh")   # col = k*NG + m*128 + j
        g_sb = const.tile([128, MT * tb], F16, tag="g")       # col = m*tb + t*16 + b
        hall = const.tile([128, KT * tb], F32, tag="hall")    # col = k*tb + t*16 + b
        xT_sb = const.tile([128, KT * tb], F32, tag="xT")     # col = k*tb + (t,b)
        bias_sb = const.tile([128, MT], F32, tag="bias")
        c_sb = const.tile([128, KT * B_LOC], F32, tag="c")    # col = (q, b)

        for k in range(KT):
            nc.sync.dma_start(wx_sb[:, k * NG:(k + 1) * NG], wx_d[k * 128:(k + 1) * 128, :])
            nc.sync.dma_start(whh_sb[:, k * NG:(k + 1) * NG], whh_d[k * 128:(k + 1) * 128, :])
            nc.sync.dma_start(xT_sb[:, k * tb:(k + 1) * tb], x_d[k * 128:(k + 1) * 128, :])
        nc.sync.dma_start(bias_sb[:], bias_d[:, :])

        if repeat > 1:
            loop_cm = tc.For_i(0, repeat, 1)
            loop_cm.__enter__()

        nc.vector.memset(c_sb[:], 0.0)

        # ---------- phase 1: G^T[m*128+p, (t,b)] = sum_f Wx[f, gate] x[(t,b), f] + b
        # 512-col groups so each Wx tile load streams 512 moving columns.
        w1 = min(512, tb)
        with tc.tile_pool(name="p1", bufs=6, space="PSUM") as p1_p:
            for cc in range(tb // w1):
                for m in range(MT):
                    ps = p1_p.tile([128, w1], F32, tag="p1")
                    for k in range(KT):
                        nc.tensor.matmul(
                            ps[:],
                            wx_sb[:, k * NG + m * 128: k * NG + (m + 1) * 128],
                            xT_sb[:, k * tb + cc * w1: k * tb + (cc + 1) * w1],
                            start=(k == 0), stop=(k == KT - 1),
                        )
                    nc.vector.tensor_scalar_add(
                        g_sb[:, m * tb + cc * w1: m * tb + (cc + 1) * w1],
                        ps[:],
                        bias_sb[:, m:m + 1],
                    )

        # ---------- scan
        g3 = g_sb[:].rearrange("p (m c) -> p m c", m=MT)
        h3 = hall[:].rearrange("p (k c) -> p k c", k=KT)
        ORDER = (2, 0, 1, 3)  # f, i, j, o
        with tc.tile_pool(name="sps", bufs=1, space="PSUM") as sps_p, \
             tc.tile_pool(name="zt", bufs=3) as z_p, \
             tc.tile_pool(name="act", bufs=3) as a_p, \
             tc.tile_pool(name="h16", bufs=3) as h_p, \
             tc.tile_pool(name="tmp", bufs=3) as tmp_p:
            hq = []
            for q in range(KT):
                t0 = h_p.tile([128, B_LOC], F16, tag="h%d" % q)
                nc.vector.memset(t0[:], 0.0)
                hq.append(t0)
            for t in range(t_steps):
                ts_ = slice(t * B_LOC, (t + 1) * B_LOC)
                # --- matmuls, half-blocked, k-outer inside each half
                pss = {}
                for Hh in (0, 1):
                    for g in ORDER:
                        pss[(g, Hh)] = sps_p.tile(
                            [128, 2 * B_LOC], F32,
                            tag="ps%d_%d" % (g, Hh), name="ps%d_%d" % (g, Hh))
                    for k in range(KT):
                        for g in ORDER:
                            for mr in (0, 1):
                                mi = 2 * Hh + mr
                                m = 4 * g + mi
                                nc.tensor.matmul(
                                    pss[(g, Hh)][:, mr * B_LOC:(mr + 1) * B_LOC],
                                    whh_sb[:, k * NG + m * 128: k * NG + (m + 1) * 128],
                                    hq[k][:],
                                    start=(k == 0 and mr == 0),
                                    stop=(k == KT - 1 and mr == 1),
                                    skip_group_check=True,
                                )
                # --- gate math per half (i/f/o merged sigmoid)
                for Hh in (0, 1):
                    mlo = 2 * Hh
                    z_ifo = z_p.tile([128, 3 * 2 * B_LOC], F32, tag="zifo%d" % Hh)
                    z_j = z_p.tile([128, 2 * B_LOC], F32, tag="zj%d" % Hh)
                    for zi, g in enumerate((0, 2, 3)):  # i, f, o zones
                        nc.vector.tensor_add(
                            z_ifo[:, zi * 32:(zi + 1) * 32].rearrange(
                                "p (m c) -> p m c", m=2),
                            pss[(g, Hh)][:].rearrange("p (m c) -> p m c", m=2),
                            g3[:, 4 * g + mlo:4 * g + mlo + 2, ts_],
                        )
                    nc.vector.tensor_add(
                        z_j[:].rearrange("p (m c) -> p m c", m=2),
                        pss[(1, Hh)][:].rearrange("p (m c) -> p m c", m=2),
                        g3[:, 4 + mlo:4 + mlo + 2, ts_],
                    )
                    sio = a_p.tile([128, 3 * 2 * B_LOC], F32, tag="sio%d" % Hh)
                    nc.scalar.activation(sio[:], z_ifo[:], AF.Sigmoid)
                    tj = a_p.tile([128, 2 * B_LOC], F32, tag="tj%d" % Hh)
                    nc.scalar.activation(tj[:], z_j[:], AF.Tanh)

                    ch = c_sb[:, Hh * 32:(Hh + 1) * 32]
                    tmp = tmp_p.tile([128, 2 * B_LOC], F32, tag="tmp%d" % Hh)
                    nc.vector.tensor_mul(tmp[:], sio[:, 0:32], tj[:])
                    nc.vector.tensor_mul(ch, ch, sio[:, 32:64])
                    nc.vector.tensor_add(ch, ch, tmp[:])
                    tanc = a_p.tile([128, 2 * B_LOC], F32, tag="tanc%d" % Hh)
                    nc.scalar.activation(tanc[:], ch, AF.Tanh)

                    hv = h3[:, mlo:mlo + 2, ts_]
                    nc.vector.tensor_mul(
                        hv,
                        tanc[:].rearrange("p (m c) -> p m c", m=2),
                        sio[:, 64:96].rearrange("p (m c) -> p m c", m=2),
                    )
                    for qr in (0, 1):
                        q = mlo + qr
                        hnew = h_p.tile([128, B_LOC], F16, tag="h%d" % q,
                                        name="hnew%d" % q)
                        nc.vector.tensor_copy(
                            hnew[:], hall[:, q * tb + t * B_LOC: q * tb + (t + 1) * B_LOC])
                        hq[q] = hnew

        nchunk = max(1, t_steps // 32)
        for k in range(KT):
            for ci in range(nchunk):
                w = tb // nchunk
                nc.sync.dma_start(
                    y_d[k * 128:(k + 1) * 128, ci * w:(ci + 1) * w],
                    hall[:, k * tb + ci * w: k * tb + (ci + 1) * w])

        if repeat > 1:
            loop_cm.__exit__(None, None, None)

    nc.compile()
    return nc


_BUILT: bass.Bass | None = None


def _get_built() -> bass.Bass:
    global _BUILT
    if _BUILT is None:
        _BUILT = build_nc(T)
    return _BUILT


def make_in_maps(x, W_fw, b_fw, W_bw, b_bw, t_steps: int = T):
    x = np.asarray(x, np.float32)
    in_maps = []
    for d, (Wd, bd) in enumerate(((W_fw, b_fw), (W_bw, b_bw))):
        Wd = np.asarray(Wd, np.float32)
        wx = np.ascontiguousarray(Wd[:F])
        whh = np.ascontiguousarray(Wd[F:]).astype(np.float16)
        bv = np.asarray(bd, np.float32).copy()
        bv[2 * H:3 * H] += FORGET_BIAS  # fold forget bias into the f-gate bias
        bias = np.ascontiguousarray(bv.reshape(MT, 128).T)
        for g in range(4):
            xg = x[g * B_LOC:(g + 1) * B_LOC, :t_steps]
            if d == 1:
                xg = xg[:, ::-1, :]
            x_t = np.ascontiguousarray(
                xg.transpose(1, 0, 2).reshape(t_steps * B_LOC, F).T
            )
            in_maps.append({"xT": x_t, "wx": wx, "whh": whh, "bias": bias})
    return in_maps


def assemble_out(results, t_steps: int = T):
    out = np.empty((B_FULL, t_steps, 2 * H), np.float32)
    for idx, r in enumerate(results):
        d, g = divmod(idx, 4)
        h = r["y"].reshape(H, t_steps, B_LOC).transpose(2, 1, 0)  # [16, T, 512]
        if d == 1:
            h = h[:, ::-1, :]
        out[g * B_LOC:(g + 1) * B_LOC, :, d * H:(d + 1) * H] = h
    return out


def kernel(x, W_fw, b_fw, W_bw, b_bw):
    from concourse.bass_utils import run_bass_kernel_spmd

    nc = _get_built()
    in_maps = make_in_maps(x, W_fw, b_fw, W_bw, b_bw)
    res = run_bass_kernel_spmd(nc, in_maps, core_ids=list(range(8)))
    return assemble_out(res.results)
